# revision 11
# baseline (speedup 1.0000x reference)
"""Classical self-attention on 8 trn2 NeuronCores.

N=16384 tokens, d=64, fp32. Sequence-parallel over Q: core c handles rows
[c*2048, (c+1)*2048). K/V computed redundantly on every core from the full x.

Math (reference):
  q = (x @ rot.T) @ Wq.T + bq = x @ (Wq@rot).T + bq
  k = x @ (Wk@ent).T + bk ;  v = x @ Wv.T + bv
  y = softmax(q @ k.T / 8) @ v

Score matmuls use a bf16 hi/lo split: k = kh + kl, q*scale = qh + ql (all
bf16).  s = kh@qh + kh@ql + kl@qh reproduces the fp32 scores to ~4e-3 abs
(the missing kl@ql term is ~2^-18 relative), at bf16 matmul speed
(1 cycle/row vs fp32's 4):
  mm1: lhsT=[kh; ones](65)   rhs=[qh; -m](65)    -> kh.qh - m
  mm2: lhsT=[kl; kh](128)    rhs=[qh; ql](128)   -> kl.qh + kh.ql
kh and ql are duplicated into the upper 64 partitions of the 128-row tiles
via SBUF->SBUF DMA (engines cannot move data across partitions).

Device layout per core:
  KH  [65,16384] bf16 : rows 0-63 = kh, row 64 = ones
  K2  [128,16384] bf16: rows 0-63 = kl, rows 64-127 = kh (DMA dup)
  QH  [65, 2048] bf16 : rows 0-63 = qh (scale+bias folded), row 64 = -rowmax
  Q2  [128, 2048] bf16: rows 0-63 = qh (DMA dup), rows 64-127 = ql
  V   [128, 128, 65] f32 : 128 token-tiles of [128 tok, 64 v | 1.0]
  pass A: s~[q,kv] chunks (bf16 kh.qh) -> PSUM -> DVE tensor_tensor_reduce
          (pairwise max of 2 chunks/op) -> rowmax -> PE-transpose -> -m
  pass B: sT[kv,q] = mm1+mm2 accumulated in PSUM -> ACT exp -> SBUF (f32r)
  PV:     OT[65,512] += V'.T @ expST  (row 64 = sum = Z)
  final:  PE-transpose OT -> y = O / Z
"""

import sys

sys.path.insert(0, "/opt/trn_rl_repo")

from contextlib import ExitStack

import numpy as np

import concourse.bass as bass
import concourse.mybir as mybir
import concourse.tile as tile
from concourse import bacc
from concourse.bass import ds, ts
from concourse.bass_utils import run_bass_kernel_spmd

N_CORES = 8
N = 16384
D = 64
QR = N // N_CORES          # 2048 q rows per core
N_QTILE = QR // 128        # 16 q tiles per core
N_GROUP = 4                # groups of 4 q-tiles (512 q cols)
TILES_PER_GROUP = N_QTILE // N_GROUP
N_KV_BLK = N // 128        # 128 kv blocks
A_CHUNK = 512              # pass-A kv chunk (1 psum bank)
N_A_CHUNKS = N // A_CHUNK  # 32 chunks per q tile
F32 = mybir.dt.float32
BF16 = mybir.dt.bfloat16
NEG_INF = -3.0e38

_CACHED = {}


def build_kernel():
    nc = bacc.Bacc("TRN2", target_bir_lowering=False, debug=False,
                   num_devices=N_CORES)

    x_d = nc.dram_tensor("x", [N, D], F32, kind="ExternalInput")
    xq_d = nc.dram_tensor("xq", [QR, D], F32, kind="ExternalInput")
    wq_d = nc.dram_tensor("wq", [D + 1, D], F32, kind="ExternalInput")
    wk_d = nc.dram_tensor("wk", [D + 1, D], F32, kind="ExternalInput")
    wv_d = nc.dram_tensor("wv", [D + 1, D], F32, kind="ExternalInput")
    id_d = nc.dram_tensor("ident", [128, 128], F32, kind="ExternalInput")
    y_d = nc.dram_tensor("y", [QR, D], F32, kind="ExternalOutput")

    R32 = mybir.dt.float32r
    MAX = mybir.AluOpType.max

    with tile.TileContext(nc) as tc, ExitStack() as ctx:
        sb = ctx.enter_context(tc.tile_pool(name="sb", bufs=1))
        xtp = ctx.enter_context(tc.tile_pool(name="xtp", bufs=2))
        expp = ctx.enter_context(tc.tile_pool(name="expp", bufs=2))
        smp = ctx.enter_context(tc.tile_pool(name="smp", bufs=4))
        psB_p = ctx.enter_context(tc.tile_pool(name="psB", bufs=1, space="PSUM"))
        psPV_p = ctx.enter_context(tc.tile_pool(name="psPV", bufs=1, space="PSUM"))
        psM_p = ctx.enter_context(tc.tile_pool(name="psM", bufs=1, space="PSUM"))

        # ---- persistent SBUF ----
        KH = sb.tile([D + 1, N], BF16)      # kh | ones row
        K2 = sb.tile([128, N], BF16)        # kl | kh (dup)
        QH = sb.tile([D + 1, QR], BF16)     # qh | -m row
        Q2 = sb.tile([128, QR], BF16)       # qh (dup) | ql
        QL = sb.tile([D, QR], BF16)         # ql staging (partitions 0-63)
        V = sb.tile([128, N_KV_BLK, D + 1], F32)
        OT = sb.tile([D + 1, QR], F32)
        y_sb = sb.tile([128, N_QTILE, D], F32)
        wq = sb.tile([D + 1, D], F32)
        wk = sb.tile([D + 1, D], F32)
        wv = sb.tile([D + 1, D], F32)
        ident = sb.tile([128, 128], F32)
        mms = sb.tile([128, N_QTILE * (N_A_CHUNKS // 2)], F32)  # pair maxes
        scr = sb.tile([128, A_CHUNK], F32)  # ttr elementwise dump (unused)

        def rPV(ap):
            return ap.bitcast(R32)

        nc.gpsimd.dma_start(wq[:], wq_d[:])
        nc.gpsimd.dma_start(wk[:], wk_d[:])
        nc.gpsimd.dma_start(wv[:], wv_d[:])
        nc.gpsimd.dma_start(ident[:], id_d[:])

        nc.gpsimd.memset(KH[D : D + 1, :], 1.0)
        nc.gpsimd.memset(V[:, :, D : D + 1], 1.0)

        # ---- PSUM tiles ----
        # Two separate half tiles so the pair-p+2 matmuls don't pick up a
        # false WAR against exp(p) via coarse whole-tile dependency tracking.
        psB0 = psB_p.tile([128, 1024], F32)            # 2 banks
        psB1 = psB_p.tile([128, 1024], F32)            # 2 banks
        psPV = psPV_p.tile([128, 512], F32)            # 1 bank
        # psM pool: tag "psm" (1 bank) + tag "psA" (2 banks) -> 8 total

        def build_xT(xT, dram_ap, ntiles):
            """dram_ap: [ntiles*128, 64] -> xT[0:D, 0:ntiles*128] via PE."""
            for h in range(0, ntiles, 16):
                nh = min(16, ntiles - h)
                xn = xtp.tile([128, 16, D], F32, tag="xn")
                nc.sync.dma_start(
                    xn[:, 0:nh, :],
                    dram_ap[ds(h * 128, nh * 128), :].rearrange(
                        "(j p) d -> p j d", p=128))
                for j4 in range(nh // 4):
                    pm = xpose_psum()
                    for jj in range(4):
                        nc.tensor.transpose(pm[:, ds(jj * 128, 128)],
                                            xn[:, j4 * 4 + jj, :], ident[:])
                    nc.scalar.copy(xT[0:D, ds(h * 128 + j4 * 512, 512)], pm[:])

        # The B-phase psum halves double as rings for the setup projection
        # outputs (psB is unused until phase 1), so the PE never stalls on
        # the split/copy reads of the previous projection: k/q projections
        # rotate psB0's halves, V projections rotate four [128,256] slots
        # of psB1.
        proj_ring = [psB0[0:D, ds(0, 512)], psB0[0:D, ds(512, 512)]]
        vproj_ring = [psB1[:, ds(j * 256, 256)] for j in range(4)]
        ring_i = [0, 0, 0]

        def proj_psum():
            pm = proj_ring[ring_i[0] % 2]
            ring_i[0] += 1
            return pm

        def vproj_psum():
            pm = vproj_ring[ring_i[1] % 4]
            ring_i[1] += 1
            return pm

        def xpose_psum():
            # alternate tags: effective 3-deep ring for the transpose quads
            ring_i[2] += 1
            if ring_i[2] % 3 == 0:
                pm = psM_p.tile([D, 512], F32, tag="psm")
            else:
                pm = psM_p.tile([128, A_CHUNK], F32, tag="psA", bufs=2)
            return pm[0:D, :]

        # ---- setup: xq -> QH/Q2/QL ----
        xqT = xtp.tile([D + 1, QR], F32, tag="xt")
        build_xT(xqT, xq_d[:], N_QTILE)
        nc.gpsimd.memset(xqT[D : D + 1, :], 1.0)
        for j in range(QR // 512):
            pm = proj_psum()
            nc.tensor.matmul(pm[:], wq[:], xqT[:, ts(j, 512)],
                             start=True, stop=True)
            nc.vector.tensor_copy(QH[0:D, ts(j, 512)], pm[:])
            nc.vector.tensor_sub(QL[:, ts(j, 512)], pm[:], QH[0:D, ts(j, 512)])
        nc.sync.dma_start(Q2[0:D, :], QH[0:D, :])
        nc.sync.dma_start(Q2[D : D + D, :], QL[:])

        # ---- setup generator: KH/K2 and V built in 4 chunks of 4096 ----
        def setup_units():
            for cc in range(4):
                xT = xtp.tile([D + 1, 4096], F32, tag="xt")
                build_xT(xT, x_d[ds(cc * 4096, 4096), :], 32)
                nc.gpsimd.memset(xT[D : D + 1, :], 1.0)
                for j in range(8):
                    pm = proj_psum()
                    nc.tensor.matmul(pm[:], wk[:], xT[:, ts(j, 512)],
                                     start=True, stop=True)
                    sl = ds(cc * 4096 + j * 512, 512)
                    nc.vector.tensor_copy(KH[0:D, sl], pm[:])
                    nc.vector.tensor_sub(K2[0:D, sl], pm[:], KH[0:D, sl])
                    yield
                nc.sync.dma_start(K2[D : D + D, ds(cc * 4096, 4096)],
                                  KH[0:D, ds(cc * 4096, 4096)])
                # V: 32 token-tiles, in quads sharing one psum slot
                for j in range(8):
                    pm = vproj_psum()
                    for jj in range(4):
                        nc.tensor.matmul(
                            pm[:, ds(jj * D, D)],
                            xT[:, ts(j * 4 + jj, 128)], wv[:],
                            start=True, stop=True)
                    b0 = cc * 32 + j * 4
                    nc.scalar.copy(rPV(V[:, ds(b0, 4), 0:D]), pm[:])
                    yield

        # ---- pass A unit: q-tile t, kv chunk c (bf16 kh.qh, max only) ----
        mfin_pending = []
        a_prev = [None]

        def flush_mfin():
            while mfin_pending:
                t, mt = mfin_pending.pop(0)
                pneg = psM_p.tile([1, 128], F32, tag="psm")
                nc.tensor.matmul(pneg[:], mt[:], ident[:], start=True, stop=True)
                # QH row 64 <- -m (bf16; per-row constant shift is exact math)
                nc.scalar.mul(QH[D : D + 1, ts(t, 128)], pneg[:], -1.0)

        def emit_A(t, c):
            if c % 8 == 4:
                flush_mfin()
            pa = psM_p.tile([128, A_CHUNK], F32, tag="psA", bufs=2)
            nc.tensor.matmul(pa[:], QH[0:D, ts(t, 128)],
                             KH[0:D, ds(c * A_CHUNK, A_CHUNK)],
                             start=True, stop=True)
            if c % 2 == 0:
                a_prev[0] = pa
            else:
                # one DVE op: elementwise max of both chunks + free-axis max
                nc.vector.tensor_tensor_reduce(
                    out=scr[:], in0=a_prev[0][:], in1=pa[:], scale=1.0,
                    scalar=NEG_INF, op0=MAX, op1=MAX,
                    accum_out=mms[:, t * 16 + c // 2 : t * 16 + c // 2 + 1])
            if c == N_A_CHUNKS - 1:
                mt = smp.tile([128, 1], F32, tag="mt")
                nc.vector.reduce_max(mt[:], mms[:, ts(t, 16)],
                                     axis=mybir.AxisListType.X)
                mfin_pending.append((t, mt))

        # ---- pass B + PV unit: group g, kv block b ----
        pv_pending = []

        def emit_PV_pending(keep=0):
            # PV(pair p) data-depends on exp(p); draining it only two pairs
            # later keeps the in-order PE queue from head-of-line blocking
            # on the ACT exp latency.
            while len(pv_pending) > keep:
                bb, ex_ap = pv_pending.pop(0)
                nc.tensor.matmul(psPV[0 : D + 1, :], rPV(V[:, bb, :]),
                                 rPV(ex_ap),
                                 start=(bb == 0), stop=(bb == N_KV_BLK - 1),
                                 skip_group_check=True)

        def emit_B(g, b):
            half = psB0 if (b // 2) % 2 == 0 else psB1
            off = (b % 2) * 512
            nc.tensor.matmul(half[:, ds(off, 512)], KH[:, ts(b, 128)],
                             QH[:, ds(g * 512, 512)], start=True, stop=False)
            nc.tensor.matmul(half[:, ds(off, 512)], K2[:, ts(b, 128)],
                             Q2[:, ds(g * 512, 512)], start=False, stop=True,
                             skip_group_check=True)
            if b % 2 == 1:
                emit_PV_pending(keep=2)
                ex = expp.tile([128, 1024], F32, tag="ex", bufs=3)
                nc.scalar.activation(rPV(ex[:]), half[:],
                                     mybir.ActivationFunctionType.Exp)
                pv_pending.append((b - 1, ex[:, ds(0, 512)]))
                pv_pending.append((b, ex[:, ds(512, 512)]))

        # ---- main pipeline ----
        # Emission order IS program order for Tile's dependency tracking.
        setup_gen = setup_units()
        setup_done = [0]

        def pace_setup(need):
            while setup_done[0] < need:
                if next(setup_gen, None) is None and setup_done[0] >= 64:
                    break
                setup_done[0] += 1

        for phase in range(N_GROUP + 1):
            a_units = []
            if phase < N_GROUP:
                for tt in range(TILES_PER_GROUP):
                    t = phase * TILES_PER_GROUP + tt
                    for c in range(N_A_CHUNKS):
                        a_units.append((t, c))
            b_units = []
            if phase > 0:
                b_units = [(phase - 1, b) for b in range(N_KV_BLK)]

            nu = max(len(a_units), len(b_units))
            for u in range(nu):
                if u < len(a_units):
                    t, c = a_units[u]
                    if phase == 0:
                        # setup unit writing KH chunk c is (c//8)*16 + c%8
                        need = 56 if t > 0 else (c // 8) * 16 + (c % 8) + 1
                        pace_setup(need)
                    emit_A(t, c)
                if u < len(b_units):
                    emit_B(*b_units[u])
            flush_mfin()
            if phase == 0:
                pace_setup(64)  # drain remaining V builds
            if phase > 0:
                g = phase - 1
                emit_PV_pending()
                nc.scalar.copy(OT[:, ds(g * 512, 512)], psPV[0 : D + 1, :])

        # ---- final: transpose OT, normalize, store ----
        for t in range(N_QTILE):
            pO = psM_p.tile([128, D + 1], F32, tag="psm")
            nc.tensor.matmul(pO[:], OT[:, ts(t, 128)],
                             ident[0 : D + 1, 0 : D + 1],
                             start=True, stop=True)
            rz = smp.tile([128, 1], F32, tag="rz")
            nc.vector.reciprocal(rz[:], pO[:, D : D + 1])
            nc.vector.tensor_scalar_mul(y_sb[:, t, :], pO[:, 0:D], rz[:])
        nc.sync.dma_start(y_d.rearrange("(t p) d -> p t d", p=128), y_sb[:])

    nc.compile()
    return nc


def _prep_inputs(x, params, Wq, bq, Wk, bk, Wv, bv):
    x = np.ascontiguousarray(x, dtype=np.float32)
    params = np.asarray(params, dtype=np.float32)
    rot = params[:, :D]
    ent = params[:, D : 2 * D]
    scale = np.float32(1.0 / np.sqrt(D))
    wq_eff = (np.asarray(Wq, np.float32) @ rot)
    wk_eff = (np.asarray(Wk, np.float32) @ ent)
    wq = np.vstack([wq_eff.T, np.asarray(bq, np.float32)[None]]) * scale
    wk = np.vstack([wk_eff.T, np.asarray(bk, np.float32)[None]])
    wv = np.vstack([np.asarray(Wv, np.float32).T,
                    np.asarray(bv, np.float32)[None]])
    ident = np.eye(128, dtype=np.float32)
    return x, np.ascontiguousarray(wq), np.ascontiguousarray(wk), \
        np.ascontiguousarray(wv), ident


def kernel(x, params, Wq, bq, Wk, bk, Wv, bv, _trace=False):
    x, wq, wk, wv, ident = _prep_inputs(x, params, Wq, bq, Wk, bk, Wv, bv)
    if "nc" not in _CACHED:
        _CACHED["nc"] = build_kernel()
    nc = _CACHED["nc"]
    in_maps = []
    for c in range(N_CORES):
        in_maps.append({
            "x": x,
            "xq": np.ascontiguousarray(x[c * QR : (c + 1) * QR]),
            "wq": wq, "wk": wk, "wv": wv, "ident": ident,
        })
    res = run_bass_kernel_spmd(nc, in_maps, core_ids=list(range(N_CORES)),
                               trace=_trace)
    out = np.concatenate([res.results[c]["y"] for c in range(N_CORES)], axis=0)
    if _trace:
        _CACHED["last_result"] = res
    global _CACHED_RES
    _CACHED_RES = res
    return out


# revision 13
# speedup vs baseline: 1.0338x; 1.0338x over previous
"""Classical self-attention on 8 trn2 NeuronCores.

N=16384 tokens, d=64, fp32. Sequence-parallel over Q: core c handles rows
[c*2048, (c+1)*2048). K/V computed redundantly on every core from the full x.

Math (reference):
  q = (x @ rot.T) @ Wq.T + bq = x @ (Wq@rot).T + bq
  k = x @ (Wk@ent).T + bk ;  v = x @ Wv.T + bv
  y = softmax(q @ k.T / 8) @ v

Score matmuls use a bf16 hi/lo split: k = kh + kl, q*scale = qh + ql (all
bf16).  s = kh@qh + kh@ql + kl@qh reproduces the fp32 scores to ~4e-3 abs
(the missing kl@ql term is ~2^-18 relative), at bf16 matmul speed
(1 cycle/row vs fp32's 4):
  mm1: lhsT=[kh; ones](65)   rhs=[qh; -m](65)    -> kh.qh - m
  mm2: lhsT=[kl; kh](128)    rhs=[qh; ql](128)   -> kl.qh + kh.ql
kh and ql are duplicated into the upper 64 partitions of the 128-row tiles
via SBUF->SBUF DMA (engines cannot move data across partitions).

Device layout per core:
  KH  [65,16384] bf16 : rows 0-63 = kh, row 64 = ones
  K2  [128,16384] bf16: rows 0-63 = kl, rows 64-127 = kh (DMA dup)
  QH  [65, 2048] bf16 : rows 0-63 = qh (scale+bias folded), row 64 = -rowmax
  Q2  [128, 2048] bf16: rows 0-63 = qh (DMA dup), rows 64-127 = ql
  V   [128, 128, 65] f32 : 128 token-tiles of [128 tok, 64 v | 1.0]
  pass A: s~[q,kv] chunks (bf16 kh.qh) -> PSUM -> DVE tensor_tensor_reduce
          (pairwise max of 2 chunks/op) -> rowmax -> PE-transpose -> -m
  pass B: sT[kv,q] = mm1+mm2 accumulated in PSUM -> ACT exp -> SBUF (f32r)
  PV:     OT[65,512] += V'.T @ expST  (row 64 = sum = Z)
  final:  PE-transpose OT -> y = O / Z
"""

import sys

sys.path.insert(0, "/opt/trn_rl_repo")

from contextlib import ExitStack

import numpy as np

import concourse.bass as bass
import concourse.mybir as mybir
import concourse.tile as tile
from concourse import bacc
from concourse.bass import ds, ts
from concourse.bass_utils import run_bass_kernel_spmd

N_CORES = 8
N = 16384
D = 64
QR = N // N_CORES          # 2048 q rows per core
N_QTILE = QR // 128        # 16 q tiles per core
N_GROUP = 4                # groups of 4 q-tiles (512 q cols)
TILES_PER_GROUP = N_QTILE // N_GROUP
N_KV_BLK = N // 128        # 128 kv blocks
A_CHUNK = 512              # pass-A kv chunk (1 psum bank)
N_A_CHUNKS = N // A_CHUNK  # 32 chunks per q tile
F32 = mybir.dt.float32
BF16 = mybir.dt.bfloat16
NEG_INF = -3.0e38

_CACHED = {}


def build_kernel():
    nc = bacc.Bacc("TRN2", target_bir_lowering=False, debug=False,
                   num_devices=N_CORES)

    x_d = nc.dram_tensor("x", [N, D], F32, kind="ExternalInput")
    xq_d = nc.dram_tensor("xq", [QR, D], F32, kind="ExternalInput")
    wq_d = nc.dram_tensor("wq", [D + 1, D], F32, kind="ExternalInput")
    wk_d = nc.dram_tensor("wk", [D + 1, D], F32, kind="ExternalInput")
    wv_d = nc.dram_tensor("wv", [D + 1, D], F32, kind="ExternalInput")
    id_d = nc.dram_tensor("ident", [128, 128], F32, kind="ExternalInput")
    y_d = nc.dram_tensor("y", [QR, D], F32, kind="ExternalOutput")

    R32 = mybir.dt.float32r
    MAX = mybir.AluOpType.max

    with tile.TileContext(nc) as tc, ExitStack() as ctx:
        sb = ctx.enter_context(tc.tile_pool(name="sb", bufs=1))
        xtp = ctx.enter_context(tc.tile_pool(name="xtp", bufs=2))
        expp = ctx.enter_context(tc.tile_pool(name="expp", bufs=2))
        smp = ctx.enter_context(tc.tile_pool(name="smp", bufs=4))
        psB_p = ctx.enter_context(tc.tile_pool(name="psB", bufs=1, space="PSUM"))
        psPV_p = ctx.enter_context(tc.tile_pool(name="psPV", bufs=1, space="PSUM"))
        psM_p = ctx.enter_context(tc.tile_pool(name="psM", bufs=1, space="PSUM"))

        # ---- persistent SBUF ----
        KH = sb.tile([D + 1, N], BF16)      # kh | ones row
        K2 = sb.tile([128, N], BF16)        # kl | kh (dup)
        QH = sb.tile([D + 1, QR], BF16)     # qh | -m row
        Q2 = sb.tile([128, QR], BF16)       # qh (dup) | ql
        QL = sb.tile([D, QR], BF16)         # ql staging (partitions 0-63)
        V = sb.tile([128, N_KV_BLK, D + 1], F32)
        OT = sb.tile([D + 1, QR], F32)
        y_sb = sb.tile([128, N_QTILE, D], F32)
        wq = sb.tile([D + 1, D], F32)
        wk = sb.tile([D + 1, D], F32)
        wv = sb.tile([D + 1, D], F32)
        ident = sb.tile([128, 128], F32)
        mms = sb.tile([128, N_QTILE * (N_A_CHUNKS // 2)], F32)  # pair maxes
        scr = sb.tile([128, A_CHUNK], F32)  # ttr elementwise dump (unused)

        def rPV(ap):
            return ap.bitcast(R32)

        nc.gpsimd.dma_start(wq[:], wq_d[:])
        nc.gpsimd.dma_start(wk[:], wk_d[:])
        nc.gpsimd.dma_start(wv[:], wv_d[:])
        nc.gpsimd.dma_start(ident[:], id_d[:])

        nc.gpsimd.memset(KH[D : D + 1, :], 1.0)
        nc.gpsimd.memset(V[:, :, D : D + 1], 1.0)

        # ---- PSUM tiles ----
        # Two separate half tiles so the pair-p+2 matmuls don't pick up a
        # false WAR against exp(p) via coarse whole-tile dependency tracking.
        psB0 = psB_p.tile([128, 1024], F32)            # 2 banks
        psB1 = psB_p.tile([128, 1024], F32)            # 2 banks
        psPV = psPV_p.tile([128, 512], F32)            # 1 bank
        # psM pool: tag "psm" (1 bank) + tag "psA" (2 banks) -> 8 total

        def build_xT(xT, dram_ap, ntiles):
            """dram_ap: [ntiles*128, 64] -> xT[0:D, 0:ntiles*128] via PE."""
            for h in range(0, ntiles, 16):
                nh = min(16, ntiles - h)
                xn = xtp.tile([128, 16, D], F32, tag="xn")
                nc.sync.dma_start(
                    xn[:, 0:nh, :],
                    dram_ap[ds(h * 128, nh * 128), :].rearrange(
                        "(j p) d -> p j d", p=128))
                for j4 in range(nh // 4):
                    pm = xpose_psum()
                    for jj in range(4):
                        nc.tensor.transpose(pm[:, ds(jj * 128, 128)],
                                            xn[:, j4 * 4 + jj, :], ident[:])
                    nc.scalar.copy(xT[0:D, ds(h * 128 + j4 * 512, 512)], pm[:])

        # During phase 0 the B-phase and PV psum tiles are idle, so setup
        # borrows them as rings: transpose quads + k/q projections share the
        # four [0:64, 512] psB quarters; V projections rotate psPV's halves.
        # The PE then never stalls on the split/copy reads of the previous
        # setup unit.
        proj_ring = [psB0[0:D, ds(0, 512)], psB0[0:D, ds(512, 512)],
                     psB1[0:D, ds(0, 512)], psB1[0:D, ds(512, 512)]]
        vproj_ring = [psPV[:, ds(0, 256)], psPV[:, ds(256, 256)]]
        ring_i = [0, 0]

        def proj_psum():
            pm = proj_ring[ring_i[0] % 4]
            ring_i[0] += 1
            return pm

        def vproj_psum():
            pm = vproj_ring[ring_i[1] % 2]
            ring_i[1] += 1
            return pm

        xpose_psum = proj_psum

        # ---- setup: xq -> QH/Q2/QL ----
        xqT = xtp.tile([D + 1, QR], F32, tag="xt")
        build_xT(xqT, xq_d[:], N_QTILE)
        nc.gpsimd.memset(xqT[D : D + 1, :], 1.0)
        for j in range(QR // 512):
            pm = proj_psum()
            nc.tensor.matmul(pm[:], wq[:], xqT[:, ts(j, 512)],
                             start=True, stop=True)
            nc.vector.tensor_copy(QH[0:D, ts(j, 512)], pm[:])
            nc.vector.tensor_sub(QL[:, ts(j, 512)], pm[:], QH[0:D, ts(j, 512)])
        nc.sync.dma_start(Q2[0:D, :], QH[0:D, :])
        nc.sync.dma_start(Q2[D : D + D, :], QL[:])

        # ---- setup generator: KH/K2 and V built in 4 chunks of 4096 ----
        def setup_units():
            for cc in range(4):
                xT = xtp.tile([D + 1, 4096], F32, tag="xt")
                build_xT(xT, x_d[ds(cc * 4096, 4096), :], 32)
                nc.gpsimd.memset(xT[D : D + 1, :], 1.0)
                for j in range(8):
                    pm = proj_psum()
                    nc.tensor.matmul(pm[:], wk[:], xT[:, ts(j, 512)],
                                     start=True, stop=True)
                    sl = ds(cc * 4096 + j * 512, 512)
                    # kh on ACT, kl on DVE: balances phase-0 engine load
                    nc.scalar.copy(KH[0:D, sl], pm[:])
                    nc.vector.tensor_sub(K2[0:D, sl], pm[:], KH[0:D, sl])
                    yield
                nc.sync.dma_start(K2[D : D + D, ds(cc * 4096, 4096)],
                                  KH[0:D, ds(cc * 4096, 4096)])
                # V: 32 token-tiles, in quads sharing one psum slot
                for j in range(8):
                    pm = vproj_psum()
                    for jj in range(4):
                        nc.tensor.matmul(
                            pm[:, ds(jj * D, D)],
                            xT[:, ts(j * 4 + jj, 128)], wv[:],
                            start=True, stop=True)
                    b0 = cc * 32 + j * 4
                    nc.scalar.copy(rPV(V[:, ds(b0, 4), 0:D]), pm[:])
                    yield

        # ---- pass A unit: q-tile t, kv chunk c (bf16 kh.qh, max only) ----
        mfin_pending = []
        a_prev = [None]

        def flush_mfin():
            while mfin_pending:
                t, mt = mfin_pending.pop(0)
                pneg = psM_p.tile([1, 128], F32, tag="psm")
                nc.tensor.matmul(pneg[:], mt[:], ident[:], start=True, stop=True)
                # QH row 64 <- -m (bf16; per-row constant shift is exact math)
                nc.scalar.mul(QH[D : D + 1, ts(t, 128)], pneg[:], -1.0)

        def emit_A(t, c):
            if c % 8 == 4:
                flush_mfin()
            pa = psM_p.tile([128, A_CHUNK], F32, tag="psA", bufs=2)
            nc.tensor.matmul(pa[:], QH[0:D, ts(t, 128)],
                             KH[0:D, ds(c * A_CHUNK, A_CHUNK)],
                             start=True, stop=True)
            if c % 2 == 0:
                a_prev[0] = pa
            else:
                # one DVE op: elementwise max of both chunks + free-axis max
                nc.vector.tensor_tensor_reduce(
                    out=scr[:], in0=a_prev[0][:], in1=pa[:], scale=1.0,
                    scalar=NEG_INF, op0=MAX, op1=MAX,
                    accum_out=mms[:, t * 16 + c // 2 : t * 16 + c // 2 + 1])
            if c == N_A_CHUNKS - 1:
                mt = smp.tile([128, 1], F32, tag="mt")
                nc.vector.reduce_max(mt[:], mms[:, ts(t, 16)],
                                     axis=mybir.AxisListType.X)
                mfin_pending.append((t, mt))

        # ---- pass B + PV unit: group g, kv block b ----
        pv_pending = []

        def emit_PV_pending(keep=0):
            # PV(pair p) data-depends on exp(p); draining it only two pairs
            # later keeps the in-order PE queue from head-of-line blocking
            # on the ACT exp latency.
            while len(pv_pending) > keep:
                bb, ex_ap = pv_pending.pop(0)
                nc.tensor.matmul(psPV[0 : D + 1, :], rPV(V[:, bb, :]),
                                 rPV(ex_ap),
                                 start=(bb == 0), stop=(bb == N_KV_BLK - 1),
                                 skip_group_check=True)

        def emit_B(g, b):
            half = psB0 if (b // 2) % 2 == 0 else psB1
            off = (b % 2) * 512
            nc.tensor.matmul(half[:, ds(off, 512)], KH[:, ts(b, 128)],
                             QH[:, ds(g * 512, 512)], start=True, stop=False)
            nc.tensor.matmul(half[:, ds(off, 512)], K2[:, ts(b, 128)],
                             Q2[:, ds(g * 512, 512)], start=False, stop=True,
                             skip_group_check=True)
            if b % 2 == 1:
                emit_PV_pending(keep=2)
                ex = expp.tile([128, 1024], F32, tag="ex", bufs=3)
                nc.scalar.activation(rPV(ex[:]), half[:],
                                     mybir.ActivationFunctionType.Exp)
                pv_pending.append((b - 1, ex[:, ds(0, 512)]))
                pv_pending.append((b, ex[:, ds(512, 512)]))

        # ---- main pipeline ----
        # Emission order IS program order for Tile's dependency tracking.
        setup_gen = setup_units()
        setup_done = [0]

        def pace_setup(need):
            while setup_done[0] < need:
                if next(setup_gen, None) is None and setup_done[0] >= 64:
                    break
                setup_done[0] += 1

        for phase in range(N_GROUP + 1):
            a_units = []
            if phase < N_GROUP:
                for tt in range(TILES_PER_GROUP):
                    t = phase * TILES_PER_GROUP + tt
                    for c in range(N_A_CHUNKS):
                        a_units.append((t, c))
            b_units = []
            if phase > 0:
                b_units = [(phase - 1, b) for b in range(N_KV_BLK)]

            nu = max(len(a_units), len(b_units))
            for u in range(nu):
                if u < len(a_units):
                    t, c = a_units[u]
                    if phase == 0:
                        # setup unit writing KH chunk c is (c//8)*16 + c%8
                        need = 56 if t > 0 else (c // 8) * 16 + (c % 8) + 1
                        pace_setup(need)
                    emit_A(t, c)
                if u < len(b_units):
                    emit_B(*b_units[u])
            flush_mfin()
            if phase == 0:
                pace_setup(64)  # drain remaining V builds
            if phase > 0:
                g = phase - 1
                emit_PV_pending()
                nc.scalar.copy(OT[:, ds(g * 512, 512)], psPV[0 : D + 1, :])

        # ---- final: transpose OT, normalize, store ----
        for t in range(N_QTILE):
            pO = psM_p.tile([128, D + 1], F32, tag="psm")
            nc.tensor.matmul(pO[:], OT[:, ts(t, 128)],
                             ident[0 : D + 1, 0 : D + 1],
                             start=True, stop=True)
            rz = smp.tile([128, 1], F32, tag="rz")
            nc.vector.reciprocal(rz[:], pO[:, D : D + 1])
            nc.vector.tensor_scalar_mul(y_sb[:, t, :], pO[:, 0:D], rz[:])
        nc.sync.dma_start(y_d.rearrange("(t p) d -> p t d", p=128), y_sb[:])

    nc.compile()
    return nc


def _prep_inputs(x, params, Wq, bq, Wk, bk, Wv, bv):
    x = np.ascontiguousarray(x, dtype=np.float32)
    params = np.asarray(params, dtype=np.float32)
    rot = params[:, :D]
    ent = params[:, D : 2 * D]
    scale = np.float32(1.0 / np.sqrt(D))
    wq_eff = (np.asarray(Wq, np.float32) @ rot)
    wk_eff = (np.asarray(Wk, np.float32) @ ent)
    wq = np.vstack([wq_eff.T, np.asarray(bq, np.float32)[None]]) * scale
    wk = np.vstack([wk_eff.T, np.asarray(bk, np.float32)[None]])
    wv = np.vstack([np.asarray(Wv, np.float32).T,
                    np.asarray(bv, np.float32)[None]])
    ident = np.eye(128, dtype=np.float32)
    return x, np.ascontiguousarray(wq), np.ascontiguousarray(wk), \
        np.ascontiguousarray(wv), ident


def kernel(x, params, Wq, bq, Wk, bk, Wv, bv, _trace=False):
    x, wq, wk, wv, ident = _prep_inputs(x, params, Wq, bq, Wk, bk, Wv, bv)
    if "nc" not in _CACHED:
        _CACHED["nc"] = build_kernel()
    nc = _CACHED["nc"]
    in_maps = []
    for c in range(N_CORES):
        in_maps.append({
            "x": x,
            "xq": np.ascontiguousarray(x[c * QR : (c + 1) * QR]),
            "wq": wq, "wk": wk, "wv": wv, "ident": ident,
        })
    res = run_bass_kernel_spmd(nc, in_maps, core_ids=list(range(N_CORES)),
                               trace=_trace)
    out = np.concatenate([res.results[c]["y"] for c in range(N_CORES)], axis=0)
    if _trace:
        _CACHED["last_result"] = res
    global _CACHED_RES
    _CACHED_RES = res
    return out


# revision 15
# speedup vs baseline: 1.1055x; 1.0693x over previous
"""Classical self-attention on 8 trn2 NeuronCores.

N=16384 tokens, d=64, fp32. Sequence-parallel over Q: core c handles rows
[c*2048, (c+1)*2048). K/V computed redundantly on every core from the full x.

Math (reference):
  q = (x @ rot.T) @ Wq.T + bq = x @ (Wq@rot).T + bq
  k = x @ (Wk@ent).T + bk ;  v = x @ Wv.T + bv
  y = softmax(q @ k.T / 8) @ v

Score matmuls use a bf16 hi/lo split: k = kh + kl, q*scale = qh + ql (all
bf16).  s = kh@qh + kh@ql + kl@qh reproduces the fp32 scores to ~4e-3 abs
(the missing kl@ql term is ~2^-18 relative), at bf16 matmul speed
(1 cycle/row vs fp32's 4):
  mm1: lhsT=[kh; ones](65)   rhs=[qh; -m](65)    -> kh.qh - m
  mm2: lhsT=[kl; kh](128)    rhs=[qh; ql](128)   -> kl.qh + kh.ql
kh and ql are duplicated into the upper 64 partitions of the 128-row tiles
via SBUF->SBUF DMA (engines cannot move data across partitions).

Device layout per core:
  KH  [65,16384] bf16 : rows 0-63 = kh, row 64 = ones
  K2  [128,16384] bf16: rows 0-63 = kl, rows 64-127 = kh (DMA dup)
  QH  [65, 2048] bf16 : rows 0-63 = qh (scale+bias folded), row 64 = -rowmax
  Q2  [128, 2048] bf16: rows 0-63 = qh (DMA dup), rows 64-127 = ql
  V   [128, 128, 65] f32 : 128 token-tiles of [128 tok, 64 v | 1.0]
  pass A: s~[q,kv] chunks (bf16 kh.qh) -> PSUM -> DVE tensor_tensor_reduce
          (pairwise max of 2 chunks/op) -> rowmax -> PE-transpose -> -m
  pass B: sT[kv,q] = mm1+mm2 accumulated in PSUM -> ACT exp -> SBUF (f32r)
  PV:     OT[65,512] += V'.T @ expST  (row 64 = sum = Z)
  final:  PE-transpose OT -> y = O / Z
"""

import sys

sys.path.insert(0, "/opt/trn_rl_repo")

from contextlib import ExitStack

import numpy as np

import concourse.bass as bass
import concourse.mybir as mybir
import concourse.tile as tile
from concourse import bacc
from concourse.bass import ds, ts
from concourse.bass_utils import run_bass_kernel_spmd

N_CORES = 8
N = 16384
D = 64
QR = N // N_CORES          # 2048 q rows per core
N_QTILE = QR // 128        # 16 q tiles per core
N_GROUP = 4                # groups of 4 q-tiles (512 q cols)
TILES_PER_GROUP = N_QTILE // N_GROUP
N_KV_BLK = N // 128        # 128 kv blocks
A_CHUNK = 512              # pass-A kv chunk (1 psum bank)
N_A_CHUNKS = N // A_CHUNK  # 32 chunks per q tile
F32 = mybir.dt.float32
BF16 = mybir.dt.bfloat16
NEG_INF = -3.0e38

_CACHED = {}


def build_kernel():
    nc = bacc.Bacc("TRN2", target_bir_lowering=False, debug=False,
                   num_devices=N_CORES)

    x_d = nc.dram_tensor("x", [N, D], F32, kind="ExternalInput")
    xq_d = nc.dram_tensor("xq", [QR, D], F32, kind="ExternalInput")
    wq_d = nc.dram_tensor("wq", [D + 1, D], F32, kind="ExternalInput")
    wk_d = nc.dram_tensor("wk", [D + 1, D], F32, kind="ExternalInput")
    wv_d = nc.dram_tensor("wv", [D + 1, D], F32, kind="ExternalInput")
    id_d = nc.dram_tensor("ident", [128, 128], F32, kind="ExternalInput")
    y_d = nc.dram_tensor("y", [QR, D], F32, kind="ExternalOutput")

    R32 = mybir.dt.float32r
    MAX = mybir.AluOpType.max

    with tile.TileContext(nc) as tc, ExitStack() as ctx:
        sb = ctx.enter_context(tc.tile_pool(name="sb", bufs=1))
        xtp = ctx.enter_context(tc.tile_pool(name="xtp", bufs=2))
        expp = ctx.enter_context(tc.tile_pool(name="expp", bufs=2))
        smp = ctx.enter_context(tc.tile_pool(name="smp", bufs=4))
        psB_p = ctx.enter_context(tc.tile_pool(name="psB", bufs=1, space="PSUM"))
        psPV_p = ctx.enter_context(tc.tile_pool(name="psPV", bufs=1, space="PSUM"))
        psM_p = ctx.enter_context(tc.tile_pool(name="psM", bufs=1, space="PSUM"))

        # ---- persistent SBUF ----
        KH = sb.tile([D + 1, N], BF16)      # kh | ones row
        K2 = sb.tile([128, N], BF16)        # kl | kh (dup)
        QH = sb.tile([D + 1, QR], BF16)     # qh | -m row
        Q2 = sb.tile([128, QR], BF16)       # qh (dup) | ql
        QL = sb.tile([D, QR], BF16)         # ql staging (partitions 0-63)
        V = sb.tile([128, N_KV_BLK, D + 1], F32)
        OT = sb.tile([D + 1, QR], F32)
        y_sb = sb.tile([128, N_QTILE, D], F32)
        wq = sb.tile([D + 1, D], F32)
        wk = sb.tile([D + 1, D], F32)
        wv = sb.tile([D + 1, D], F32)
        ident = sb.tile([128, 128], F32)
        mms = sb.tile([128, N_QTILE * (N_A_CHUNKS // 2)], F32)  # pair maxes
        scr = sb.tile([128, A_CHUNK], F32)  # ttr elementwise dump (unused)

        def rPV(ap):
            return ap.bitcast(R32)

        nc.gpsimd.dma_start(ident[:], id_d[:])
        nc.gpsimd.dma_start(wq[:], wq_d[:])
        nc.gpsimd.dma_start(wk[:], wk_d[:])
        nc.gpsimd.dma_start(wv[:], wv_d[:])

        nc.vector.memset(V[:, :, D : D + 1], 1.0)

        # ---- PSUM tiles ----
        # Two separate half tiles so the pair-p+2 matmuls don't pick up a
        # false WAR against exp(p) via coarse whole-tile dependency tracking.
        psB0 = psB_p.tile([128, 1024], F32)            # 2 banks
        psB1 = psB_p.tile([128, 1024], F32)            # 2 banks
        psPV = psPV_p.tile([128, 512], F32)            # 1 bank
        # psM pool: tag "psm" (1 bank) + tag "psA" (2 banks) -> 8 total

        def build_xT(xT, dram_ap, ntiles):
            """dram_ap: [ntiles*128, 64] -> xT[0:D, 0:ntiles*128] via PE."""
            for h in range(0, ntiles, 16):
                nh = min(16, ntiles - h)
                xn = xtp.tile([128, 16, D], F32, tag="xn")
                nc.sync.dma_start(
                    xn[:, 0:nh, :],
                    dram_ap[ds(h * 128, nh * 128), :].rearrange(
                        "(j p) d -> p j d", p=128))
                for j4 in range(nh // 4):
                    pm = xpose_psum()
                    for jj in range(4):
                        nc.tensor.transpose(pm[:, ds(jj * 128, 128)],
                                            xn[:, j4 * 4 + jj, :], ident[:])
                    nc.scalar.copy(xT[0:D, ds(h * 128 + j4 * 512, 512)], pm[:])

        # During phase 0 the B-phase and PV psum tiles are idle, so setup
        # borrows them as rings: transpose quads + k/q projections share the
        # four [0:64, 512] psB quarters; V projections rotate psPV's halves.
        # The PE then never stalls on the split/copy reads of the previous
        # setup unit.
        proj_ring = [psB0[0:D, ds(0, 512)], psB0[0:D, ds(512, 512)],
                     psB1[0:D, ds(0, 512)], psB1[0:D, ds(512, 512)]]
        vproj_ring = [psPV[:, ds(0, 256)], psPV[:, ds(256, 256)]]
        ring_i = [0, 0]

        def proj_psum():
            pm = proj_ring[ring_i[0] % 4]
            ring_i[0] += 1
            return pm

        def vproj_psum():
            pm = vproj_ring[ring_i[1] % 2]
            ring_i[1] += 1
            return pm

        xpose_psum = proj_psum

        # ---- setup: xq -> QH/Q2/QL ----
        xqT = xtp.tile([D + 1, QR], F32, tag="xt")
        build_xT(xqT, xq_d[:], N_QTILE)
        nc.gpsimd.memset(xqT[D : D + 1, :], 1.0)
        for j in range(QR // 512):
            pm = proj_psum()
            nc.tensor.matmul(pm[:], wq[:], xqT[:, ts(j, 512)],
                             start=True, stop=True)
            nc.vector.tensor_copy(QH[0:D, ts(j, 512)], pm[:])
            nc.vector.tensor_sub(QL[:, ts(j, 512)], pm[:], QH[0:D, ts(j, 512)])
        nc.gpsimd.dma_start(Q2[0:D, :], QH[0:D, :])
        nc.gpsimd.dma_start(Q2[D : D + D, :], QL[:])

        # ---- setup generator: KH/K2 and V built in 4 chunks of 4096 ----
        def setup_units():
            for cc in range(4):
                xT = xtp.tile([D + 1, 4096], F32, tag="xt")
                build_xT(xT, x_d[ds(cc * 4096, 4096), :], 32)
                nc.gpsimd.memset(xT[D : D + 1, :], 1.0)
                for j in range(8):
                    pm = proj_psum()
                    nc.tensor.matmul(pm[:], wk[:], xT[:, ts(j, 512)],
                                     start=True, stop=True)
                    sl = ds(cc * 4096 + j * 512, 512)
                    # kh on ACT, kl on DVE: balances phase-0 engine load
                    nc.scalar.copy(KH[0:D, sl], pm[:])
                    nc.vector.tensor_sub(K2[0:D, sl], pm[:], KH[0:D, sl])
                    yield
                nc.gpsimd.memset(KH[D : D + 1, ds(cc * 4096, 4096)], 1.0)
                nc.gpsimd.dma_start(K2[D : D + D, ds(cc * 4096, 4096)],
                                  KH[0:D, ds(cc * 4096, 4096)])
                # V: 32 token-tiles, in quads sharing one psum slot
                for j in range(8):
                    pm = vproj_psum()
                    for jj in range(4):
                        nc.tensor.matmul(
                            pm[:, ds(jj * D, D)],
                            xT[:, ts(j * 4 + jj, 128)], wv[:],
                            start=True, stop=True)
                    b0 = cc * 32 + j * 4
                    nc.scalar.copy(rPV(V[:, ds(b0, 4), 0:D]), pm[:])
                    yield

        # ---- pass A unit: q-tile t, kv chunk c (bf16 kh.qh, max only) ----
        mfin_pending = []
        a_prev = [None]

        def flush_mfin():
            while mfin_pending:
                t, mt = mfin_pending.pop(0)
                pneg = psM_p.tile([1, 128], F32, tag="psm")
                nc.tensor.matmul(pneg[:], mt[:], ident[:], start=True, stop=True)
                # QH row 64 <- -m (bf16; per-row constant shift is exact math)
                nc.scalar.mul(QH[D : D + 1, ts(t, 128)], pneg[:], -1.0)

        def emit_A(t, c):
            if c % 8 == 4:
                flush_mfin()
            pa = psM_p.tile([128, A_CHUNK], F32, tag="psA", bufs=2)
            nc.tensor.matmul(pa[:], QH[0:D, ts(t, 128)],
                             KH[0:D, ds(c * A_CHUNK, A_CHUNK)],
                             start=True, stop=True)
            if c % 2 == 0:
                a_prev[0] = pa
            else:
                # one DVE op: elementwise max of both chunks + free-axis max
                nc.vector.tensor_tensor_reduce(
                    out=scr[:], in0=a_prev[0][:], in1=pa[:], scale=1.0,
                    scalar=NEG_INF, op0=MAX, op1=MAX,
                    accum_out=mms[:, t * 16 + c // 2 : t * 16 + c // 2 + 1])
            if c == N_A_CHUNKS - 1:
                mt = smp.tile([128, 1], F32, tag="mt")
                nc.vector.reduce_max(mt[:], mms[:, ts(t, 16)],
                                     axis=mybir.AxisListType.X)
                mfin_pending.append((t, mt))

        # ---- pass B + PV unit: group g, kv block b ----
        pv_pending = []

        def emit_PV_pending(keep=0):
            # PV(pair p) data-depends on exp(p); draining it only two pairs
            # later keeps the in-order PE queue from head-of-line blocking
            # on the ACT exp latency.
            while len(pv_pending) > keep:
                bb, ex_ap = pv_pending.pop(0)
                nc.tensor.matmul(psPV[0 : D + 1, :], rPV(V[:, bb, :]),
                                 rPV(ex_ap),
                                 start=(bb == 0), stop=(bb == N_KV_BLK - 1),
                                 skip_group_check=True)

        def emit_B(g, b):
            half = psB0 if (b // 2) % 2 == 0 else psB1
            off = (b % 2) * 512
            nc.tensor.matmul(half[:, ds(off, 512)], KH[:, ts(b, 128)],
                             QH[:, ds(g * 512, 512)], start=True, stop=False)
            nc.tensor.matmul(half[:, ds(off, 512)], K2[:, ts(b, 128)],
                             Q2[:, ds(g * 512, 512)], start=False, stop=True,
                             skip_group_check=True)
            if b % 2 == 1:
                emit_PV_pending(keep=2)
                ex = expp.tile([128, 1024], F32, tag="ex", bufs=3)
                nc.scalar.activation(rPV(ex[:]), half[:],
                                     mybir.ActivationFunctionType.Exp)
                pv_pending.append((b - 1, ex[:, ds(0, 512)]))
                pv_pending.append((b, ex[:, ds(512, 512)]))

        # ---- main pipeline ----
        # Emission order IS program order for Tile's dependency tracking.
        setup_gen = setup_units()
        setup_done = [0]

        def pace_setup(need):
            while setup_done[0] < need:
                if next(setup_gen, None) is None and setup_done[0] >= 64:
                    break
                setup_done[0] += 1

        final_pending = []

        def emit_final():
            t = final_pending.pop(0)
            pO = psM_p.tile([128, D + 1], F32, tag="psm")
            nc.tensor.matmul(pO[:], OT[:, ts(t, 128)],
                             ident[0 : D + 1, 0 : D + 1],
                             start=True, stop=True)
            rz = smp.tile([128, 1], F32, tag="rz")
            nc.vector.reciprocal(rz[:], pO[:, D : D + 1])
            nc.vector.tensor_scalar_mul(y_sb[:, t, :], pO[:, 0:D], rz[:])

        for phase in range(N_GROUP + 1):
            # pair-major A order: both chunks of a ttr pair for tile t, then
            # the next tile -- spreads the DVE max-scan evenly through the
            # phase instead of leaving a scan-bound tail.
            a_units = []
            if phase < N_GROUP:
                for c2 in range(N_A_CHUNKS // 2):
                    for tt in range(TILES_PER_GROUP):
                        t = phase * TILES_PER_GROUP + tt
                        a_units.append((t, 2 * c2, c2 * 4 + tt))
                        a_units.append((t, 2 * c2 + 1, c2 * 4 + tt))
            b_units = []
            if phase > 0:
                b_units = [(phase - 1, b) for b in range(N_KV_BLK)]

            nu = max(len(a_units), len(b_units))
            for u in range(nu):
                if u < len(a_units):
                    t, c, spread = a_units[u]
                    if phase == 0:
                        # setup unit writing KH chunk c is (c//8)*16 + c%8;
                        # spread consumes the 64 setup units evenly.
                        cov = (c // 8) * 16 + (c % 8) + 1
                        pace_setup(max(cov, min(64, spread + 1)))
                    emit_A(t, c)
                if u < len(b_units):
                    emit_B(*b_units[u])
                if final_pending and u >= 8:
                    emit_final()
            flush_mfin()
            if phase == 0:
                pace_setup(64)  # drain remaining V builds
            if phase > 0:
                g = phase - 1
                emit_PV_pending()
                nc.scalar.copy(OT[:, ds(g * 512, 512)], psPV[0 : D + 1, :])
                final_pending.extend(
                    g * TILES_PER_GROUP + tt for tt in range(TILES_PER_GROUP))

        while final_pending:
            emit_final()
        nc.sync.dma_start(y_d.rearrange("(t p) d -> p t d", p=128), y_sb[:])

    nc.compile()
    return nc


def _prep_inputs(x, params, Wq, bq, Wk, bk, Wv, bv):
    x = np.ascontiguousarray(x, dtype=np.float32)
    params = np.asarray(params, dtype=np.float32)
    rot = params[:, :D]
    ent = params[:, D : 2 * D]
    scale = np.float32(1.0 / np.sqrt(D))
    wq_eff = (np.asarray(Wq, np.float32) @ rot)
    wk_eff = (np.asarray(Wk, np.float32) @ ent)
    wq = np.vstack([wq_eff.T, np.asarray(bq, np.float32)[None]]) * scale
    wk = np.vstack([wk_eff.T, np.asarray(bk, np.float32)[None]])
    wv = np.vstack([np.asarray(Wv, np.float32).T,
                    np.asarray(bv, np.float32)[None]])
    ident = np.eye(128, dtype=np.float32)
    return x, np.ascontiguousarray(wq), np.ascontiguousarray(wk), \
        np.ascontiguousarray(wv), ident


def kernel(x, params, Wq, bq, Wk, bk, Wv, bv, _trace=False):
    x, wq, wk, wv, ident = _prep_inputs(x, params, Wq, bq, Wk, bk, Wv, bv)
    if "nc" not in _CACHED:
        _CACHED["nc"] = build_kernel()
    nc = _CACHED["nc"]
    in_maps = []
    for c in range(N_CORES):
        in_maps.append({
            "x": x,
            "xq": np.ascontiguousarray(x[c * QR : (c + 1) * QR]),
            "wq": wq, "wk": wk, "wv": wv, "ident": ident,
        })
    res = run_bass_kernel_spmd(nc, in_maps, core_ids=list(range(N_CORES)),
                               trace=_trace)
    out = np.concatenate([res.results[c]["y"] for c in range(N_CORES)], axis=0)
    if _trace:
        _CACHED["last_result"] = res
    global _CACHED_RES
    _CACHED_RES = res
    return out


# revision 16
# speedup vs baseline: 1.1061x; 1.0006x over previous
"""Classical self-attention on 8 trn2 NeuronCores.

N=16384 tokens, d=64, fp32. Sequence-parallel over Q: core c handles rows
[c*2048, (c+1)*2048). K/V computed redundantly on every core from the full x.

Math (reference):
  q = (x @ rot.T) @ Wq.T + bq = x @ (Wq@rot).T + bq
  k = x @ (Wk@ent).T + bk ;  v = x @ Wv.T + bv
  y = softmax(q @ k.T / 8) @ v

Score matmuls use a bf16 hi/lo split: k = kh + kl, q*scale = qh + ql (all
bf16).  s = kh@qh + kh@ql + kl@qh reproduces the fp32 scores to ~4e-3 abs
(the missing kl@ql term is ~2^-18 relative), at bf16 matmul speed
(1 cycle/row vs fp32's 4):
  mm1: lhsT=[kh; ones](65)   rhs=[qh; -m](65)    -> kh.qh - m
  mm2: lhsT=[kl; kh](128)    rhs=[qh; ql](128)   -> kl.qh + kh.ql
kh and ql are duplicated into the upper 64 partitions of the 128-row tiles
via SBUF->SBUF DMA (engines cannot move data across partitions).

Device layout per core:
  KH  [65,16384] bf16 : rows 0-63 = kh, row 64 = ones
  K2  [128,16384] bf16: rows 0-63 = kl, rows 64-127 = kh (DMA dup)
  QH  [65, 2048] bf16 : rows 0-63 = qh (scale+bias folded), row 64 = -rowmax
  Q2  [128, 2048] bf16: rows 0-63 = qh (DMA dup), rows 64-127 = ql
  V   [128, 128, 65] f32 : 128 token-tiles of [128 tok, 64 v | 1.0]
  pass A: s~[q,kv] chunks (bf16 kh.qh) -> PSUM -> DVE tensor_tensor_reduce
          (pairwise max of 2 chunks/op) -> rowmax -> PE-transpose -> -m
  pass B: sT[kv,q] = mm1+mm2 accumulated in PSUM -> ACT exp -> SBUF (f32r)
  PV:     OT[65,512] += V'.T @ expST  (row 64 = sum = Z)
  final:  PE-transpose OT -> y = O / Z
"""

import sys

sys.path.insert(0, "/opt/trn_rl_repo")

from contextlib import ExitStack

import numpy as np

import concourse.bass as bass
import concourse.mybir as mybir
import concourse.tile as tile
from concourse import bacc
from concourse.bass import ds, ts
from concourse.bass_utils import run_bass_kernel_spmd

N_CORES = 8
N = 16384
D = 64
QR = N // N_CORES          # 2048 q rows per core
N_QTILE = QR // 128        # 16 q tiles per core
N_GROUP = 4                # groups of 4 q-tiles (512 q cols)
TILES_PER_GROUP = N_QTILE // N_GROUP
N_KV_BLK = N // 128        # 128 kv blocks
A_CHUNK = 512              # pass-A kv chunk (1 psum bank)
N_A_CHUNKS = N // A_CHUNK  # 32 chunks per q tile
F32 = mybir.dt.float32
BF16 = mybir.dt.bfloat16
NEG_INF = -3.0e38

_CACHED = {}


def build_kernel():
    nc = bacc.Bacc("TRN2", target_bir_lowering=False, debug=False,
                   num_devices=N_CORES)

    x_d = nc.dram_tensor("x", [N, D], F32, kind="ExternalInput")
    xq_d = nc.dram_tensor("xq", [QR, D], F32, kind="ExternalInput")
    wq_d = nc.dram_tensor("wq", [D + 1, D], F32, kind="ExternalInput")
    wk_d = nc.dram_tensor("wk", [D + 1, D], F32, kind="ExternalInput")
    wv_d = nc.dram_tensor("wv", [D + 1, D], F32, kind="ExternalInput")
    id_d = nc.dram_tensor("ident", [128, 128], F32, kind="ExternalInput")
    y_d = nc.dram_tensor("y", [QR, D], F32, kind="ExternalOutput")

    R32 = mybir.dt.float32r
    MAX = mybir.AluOpType.max

    with tile.TileContext(nc) as tc, ExitStack() as ctx:
        sb = ctx.enter_context(tc.tile_pool(name="sb", bufs=1))
        xtp = ctx.enter_context(tc.tile_pool(name="xtp", bufs=2))
        expp = ctx.enter_context(tc.tile_pool(name="expp", bufs=2))
        smp = ctx.enter_context(tc.tile_pool(name="smp", bufs=4))
        psB_p = ctx.enter_context(tc.tile_pool(name="psB", bufs=1, space="PSUM"))
        psPV_p = ctx.enter_context(tc.tile_pool(name="psPV", bufs=1, space="PSUM"))
        psM_p = ctx.enter_context(tc.tile_pool(name="psM", bufs=1, space="PSUM"))

        # ---- persistent SBUF ----
        KH = sb.tile([D + 1, N], BF16)      # kh | ones row
        K2 = sb.tile([128, N], BF16)        # kl | kh (dup)
        QH = sb.tile([D + 1, QR], BF16)     # qh | -m row
        Q2 = sb.tile([128, QR], BF16)       # qh (dup) | ql
        QL = sb.tile([D, QR], BF16)         # ql staging (partitions 0-63)
        V = sb.tile([128, N_KV_BLK, D + 1], F32)
        OT = sb.tile([D + 1, QR], F32)
        y_sb = sb.tile([128, N_QTILE, D], F32)
        wq = sb.tile([D + 1, D], F32)
        wk = sb.tile([D + 1, D], F32)
        wv = sb.tile([D + 1, D], F32)
        ident = sb.tile([128, 128], F32)
        mms = sb.tile([128, N_QTILE * (N_A_CHUNKS // 2)], F32)  # pair maxes
        scr = sb.tile([128, A_CHUNK], F32)  # ttr elementwise dump (unused)

        def rPV(ap):
            return ap.bitcast(R32)

        nc.gpsimd.dma_start(ident[:], id_d[:])
        nc.gpsimd.dma_start(wq[:], wq_d[:])
        nc.gpsimd.dma_start(wk[:], wk_d[:])
        nc.gpsimd.dma_start(wv[:], wv_d[:])

        nc.vector.memset(V[:, :, D : D + 1], 1.0)

        # ---- PSUM tiles ----
        # Two separate half tiles so the pair-p+2 matmuls don't pick up a
        # false WAR against exp(p) via coarse whole-tile dependency tracking.
        psB0 = psB_p.tile([128, 1024], F32)            # 2 banks
        psB1 = psB_p.tile([128, 1024], F32)            # 2 banks
        psPV = psPV_p.tile([128, 512], F32)            # 1 bank
        # psM pool: tag "psm" (1 bank) + tag "psA" (2 banks) -> 8 total

        def build_xT(xT, dram_ap, ntiles, first_small=False):
            """dram_ap: [ntiles*128, 64] -> xT[0:D, 0:ntiles*128] via PE."""
            splits = list(range(0, ntiles, 16))
            if first_small:
                splits = [0, 4] + [h for h in splits if h >= 16]
            for i, h in enumerate(splits):
                nxt = splits[i + 1] if i + 1 < len(splits) else ntiles
                nh = nxt - h
                xn = xtp.tile([128, 16, D], F32, tag="xn")
                nc.sync.dma_start(
                    xn[:, 0:nh, :],
                    dram_ap[ds(h * 128, nh * 128), :].rearrange(
                        "(j p) d -> p j d", p=128))
                for j4 in range(nh // 4):
                    pm = xpose_psum()
                    for jj in range(4):
                        nc.tensor.transpose(pm[:, ds(jj * 128, 128)],
                                            xn[:, j4 * 4 + jj, :], ident[:])
                    nc.scalar.copy(xT[0:D, ds(h * 128 + j4 * 512, 512)], pm[:])

        # During phase 0 the B-phase and PV psum tiles are idle, so setup
        # borrows them as rings: transpose quads + k/q projections share the
        # four [0:64, 512] psB quarters; V projections rotate psPV's halves.
        # The PE then never stalls on the split/copy reads of the previous
        # setup unit.
        proj_ring = [psB0[0:D, ds(0, 512)], psB0[0:D, ds(512, 512)],
                     psB1[0:D, ds(0, 512)], psB1[0:D, ds(512, 512)]]
        vproj_ring = [psPV[:, ds(0, 256)], psPV[:, ds(256, 256)]]
        ring_i = [0, 0]

        def proj_psum():
            pm = proj_ring[ring_i[0] % 4]
            ring_i[0] += 1
            return pm

        def vproj_psum():
            pm = vproj_ring[ring_i[1] % 2]
            ring_i[1] += 1
            return pm

        xpose_psum = proj_psum

        # ---- setup: xq -> QH/Q2/QL ----
        xqT = xtp.tile([D + 1, QR], F32, tag="xt")
        build_xT(xqT, xq_d[:], N_QTILE, first_small=True)
        nc.gpsimd.memset(xqT[D : D + 1, :], 1.0)
        for j in range(QR // 512):
            pm = proj_psum()
            nc.tensor.matmul(pm[:], wq[:], xqT[:, ts(j, 512)],
                             start=True, stop=True)
            nc.vector.tensor_copy(QH[0:D, ts(j, 512)], pm[:])
            nc.vector.tensor_sub(QL[:, ts(j, 512)], pm[:], QH[0:D, ts(j, 512)])
        nc.gpsimd.dma_start(Q2[0:D, :], QH[0:D, :])
        nc.gpsimd.dma_start(Q2[D : D + D, :], QL[:])

        # ---- setup generator: KH/K2 and V built in 4 chunks of 4096 ----
        def setup_units():
            for cc in range(4):
                xT = xtp.tile([D + 1, 4096], F32, tag="xt")
                build_xT(xT, x_d[ds(cc * 4096, 4096), :], 32)
                nc.gpsimd.memset(xT[D : D + 1, :], 1.0)
                for j in range(8):
                    # k-unit (even positions, so pass-A cov pacing is even)
                    pm = proj_psum()
                    nc.tensor.matmul(pm[:], wk[:], xT[:, ts(j, 512)],
                                     start=True, stop=True)
                    sl = ds(cc * 4096 + j * 512, 512)
                    # kh on ACT, kl on DVE: balances phase-0 engine load
                    nc.scalar.copy(KH[0:D, sl], pm[:])
                    nc.vector.tensor_sub(K2[0:D, sl], pm[:], KH[0:D, sl])
                    yield
                    # v-unit: quad of token-tiles sharing one psum slot
                    pm = vproj_psum()
                    for jj in range(4):
                        nc.tensor.matmul(
                            pm[:, ds(jj * D, D)],
                            xT[:, ts(j * 4 + jj, 128)], wv[:],
                            start=True, stop=True)
                    b0 = cc * 32 + j * 4
                    nc.scalar.copy(rPV(V[:, ds(b0, 4), 0:D]), pm[:])
                    yield
                nc.gpsimd.memset(KH[D : D + 1, ds(cc * 4096, 4096)], 1.0)
                nc.gpsimd.dma_start(K2[D : D + D, ds(cc * 4096, 4096)],
                                  KH[0:D, ds(cc * 4096, 4096)])

        # ---- pass A unit: q-tile t, kv chunk c (bf16 kh.qh, max only) ----
        mfin_pending = []
        a_prev = [None]

        def flush_mfin():
            while mfin_pending:
                t, mt = mfin_pending.pop(0)
                pneg = psM_p.tile([1, 128], F32, tag="psm")
                nc.tensor.matmul(pneg[:], mt[:], ident[:], start=True, stop=True)
                # QH row 64 <- -m (bf16; per-row constant shift is exact math)
                nc.scalar.mul(QH[D : D + 1, ts(t, 128)], pneg[:], -1.0)

        def emit_A(t, c):
            if c % 8 == 4:
                flush_mfin()
            pa = psM_p.tile([128, A_CHUNK], F32, tag="psA", bufs=2)
            nc.tensor.matmul(pa[:], QH[0:D, ts(t, 128)],
                             KH[0:D, ds(c * A_CHUNK, A_CHUNK)],
                             start=True, stop=True)
            if c % 2 == 0:
                a_prev[0] = pa
            else:
                # one DVE op: elementwise max of both chunks + free-axis max
                nc.vector.tensor_tensor_reduce(
                    out=scr[:], in0=a_prev[0][:], in1=pa[:], scale=1.0,
                    scalar=NEG_INF, op0=MAX, op1=MAX,
                    accum_out=mms[:, t * 16 + c // 2 : t * 16 + c // 2 + 1])
            if c == N_A_CHUNKS - 1:
                mt = smp.tile([128, 1], F32, tag="mt")
                nc.vector.reduce_max(mt[:], mms[:, ts(t, 16)],
                                     axis=mybir.AxisListType.X)
                mfin_pending.append((t, mt))

        # ---- pass B + PV unit: group g, kv block b ----
        pv_pending = []

        def emit_PV_pending(keep=0):
            # PV(pair p) data-depends on exp(p); draining it only two pairs
            # later keeps the in-order PE queue from head-of-line blocking
            # on the ACT exp latency.
            while len(pv_pending) > keep:
                bb, ex_ap = pv_pending.pop(0)
                nc.tensor.matmul(psPV[0 : D + 1, :], rPV(V[:, bb, :]),
                                 rPV(ex_ap),
                                 start=(bb == 0), stop=(bb == N_KV_BLK - 1),
                                 skip_group_check=True)

        def emit_B(g, b):
            half = psB0 if (b // 2) % 2 == 0 else psB1
            off = (b % 2) * 512
            nc.tensor.matmul(half[:, ds(off, 512)], KH[:, ts(b, 128)],
                             QH[:, ds(g * 512, 512)], start=True, stop=False)
            nc.tensor.matmul(half[:, ds(off, 512)], K2[:, ts(b, 128)],
                             Q2[:, ds(g * 512, 512)], start=False, stop=True,
                             skip_group_check=True)
            if b % 2 == 1:
                emit_PV_pending(keep=2)
                ex = expp.tile([128, 1024], F32, tag="ex", bufs=3)
                nc.scalar.activation(rPV(ex[:]), half[:],
                                     mybir.ActivationFunctionType.Exp)
                pv_pending.append((b - 1, ex[:, ds(0, 512)]))
                pv_pending.append((b, ex[:, ds(512, 512)]))

        # ---- main pipeline ----
        # Emission order IS program order for Tile's dependency tracking.
        setup_gen = setup_units()
        setup_done = [0]

        def pace_setup(need):
            while setup_done[0] < need:
                if next(setup_gen, None) is None and setup_done[0] >= 64:
                    break
                setup_done[0] += 1

        final_pending = []

        def emit_final():
            t = final_pending.pop(0)
            pO = psM_p.tile([128, D + 1], F32, tag="psm")
            nc.tensor.matmul(pO[:], OT[:, ts(t, 128)],
                             ident[0 : D + 1, 0 : D + 1],
                             start=True, stop=True)
            rz = smp.tile([128, 1], F32, tag="rz")
            nc.vector.reciprocal(rz[:], pO[:, D : D + 1])
            nc.vector.tensor_scalar_mul(y_sb[:, t, :], pO[:, 0:D], rz[:])

        for phase in range(N_GROUP + 1):
            # pair-major A order: both chunks of a ttr pair for tile t, then
            # the next tile -- spreads the DVE max-scan evenly through the
            # phase instead of leaving a scan-bound tail.
            a_units = []
            if phase < N_GROUP:
                for c2 in range(N_A_CHUNKS // 2):
                    for tt in range(TILES_PER_GROUP):
                        t = phase * TILES_PER_GROUP + tt
                        a_units.append((t, 2 * c2, c2 * 4 + tt))
                        a_units.append((t, 2 * c2 + 1, c2 * 4 + tt))
            b_units = []
            if phase > 0:
                b_units = [(phase - 1, b) for b in range(N_KV_BLK)]

            nu = max(len(a_units), len(b_units))
            for u in range(nu):
                if u < len(a_units):
                    t, c, spread = a_units[u]
                    if phase == 0:
                        # setup unit writing KH chunk c is (c//8)*16+2*(c%8);
                        # spread consumes the 64 setup units evenly.
                        cov = (c // 8) * 16 + 2 * (c % 8) + 1
                        pace_setup(max(cov, min(64, spread + 1)))
                    emit_A(t, c)
                if u < len(b_units):
                    emit_B(*b_units[u])
                if final_pending and u >= 8:
                    emit_final()
            flush_mfin()
            if phase == 0:
                pace_setup(64)  # drain remaining V builds
            if phase > 0:
                g = phase - 1
                emit_PV_pending()
                nc.scalar.copy(OT[:, ds(g * 512, 512)], psPV[0 : D + 1, :])
                final_pending.extend(
                    g * TILES_PER_GROUP + tt for tt in range(TILES_PER_GROUP))
                if g > 0:
                    gp = g - 1
                    nc.sync.dma_start(
                        y_d.rearrange("(t p) d -> p t d", p=128)[
                            :, ds(gp * TILES_PER_GROUP, TILES_PER_GROUP), :],
                        y_sb[:, ds(gp * TILES_PER_GROUP, TILES_PER_GROUP), :])

        while final_pending:
            emit_final()
        nc.sync.dma_start(
            y_d.rearrange("(t p) d -> p t d", p=128)[
                :, ds(3 * TILES_PER_GROUP, TILES_PER_GROUP), :],
            y_sb[:, ds(3 * TILES_PER_GROUP, TILES_PER_GROUP), :])

    nc.compile()
    return nc


def _prep_inputs(x, params, Wq, bq, Wk, bk, Wv, bv):
    x = np.ascontiguousarray(x, dtype=np.float32)
    params = np.asarray(params, dtype=np.float32)
    rot = params[:, :D]
    ent = params[:, D : 2 * D]
    scale = np.float32(1.0 / np.sqrt(D))
    wq_eff = (np.asarray(Wq, np.float32) @ rot)
    wk_eff = (np.asarray(Wk, np.float32) @ ent)
    wq = np.vstack([wq_eff.T, np.asarray(bq, np.float32)[None]]) * scale
    wk = np.vstack([wk_eff.T, np.asarray(bk, np.float32)[None]])
    wv = np.vstack([np.asarray(Wv, np.float32).T,
                    np.asarray(bv, np.float32)[None]])
    ident = np.eye(128, dtype=np.float32)
    return x, np.ascontiguousarray(wq), np.ascontiguousarray(wk), \
        np.ascontiguousarray(wv), ident


def kernel(x, params, Wq, bq, Wk, bk, Wv, bv, _trace=False):
    x, wq, wk, wv, ident = _prep_inputs(x, params, Wq, bq, Wk, bk, Wv, bv)
    if "nc" not in _CACHED:
        _CACHED["nc"] = build_kernel()
    nc = _CACHED["nc"]
    in_maps = []
    for c in range(N_CORES):
        in_maps.append({
            "x": x,
            "xq": np.ascontiguousarray(x[c * QR : (c + 1) * QR]),
            "wq": wq, "wk": wk, "wv": wv, "ident": ident,
        })
    res = run_bass_kernel_spmd(nc, in_maps, core_ids=list(range(N_CORES)),
                               trace=_trace)
    out = np.concatenate([res.results[c]["y"] for c in range(N_CORES)], axis=0)
    if _trace:
        _CACHED["last_result"] = res
    global _CACHED_RES
    _CACHED_RES = res
    return out


# revision 18
# speedup vs baseline: 1.1370x; 1.0279x over previous
"""Classical self-attention on 8 trn2 NeuronCores.

N=16384 tokens, d=64, fp32. Sequence-parallel over Q: core c handles rows
[c*2048, (c+1)*2048). K/V computed redundantly on every core from the full x.

Math (reference):
  q = (x @ rot.T) @ Wq.T + bq = x @ (Wq@rot).T + bq
  k = x @ (Wk@ent).T + bk ;  v = x @ Wv.T + bv
  y = softmax(q @ k.T / 8) @ v

Score matmuls use a bf16 hi/lo split: k = kh + kl, q*scale = qh + ql (all
bf16).  s = kh@qh + kh@ql + kl@qh reproduces the fp32 scores to ~4e-3 abs
(the missing kl@ql term is ~2^-18 relative), at bf16 matmul speed
(1 cycle/row vs fp32's 4):
  mm1: lhsT=[kh; ones](65)   rhs=[qh; -m](65)    -> kh.qh - m
  mm2: lhsT=[kl; kh](128)    rhs=[qh; ql](128)   -> kl.qh + kh.ql
kh and ql are duplicated into the upper 64 partitions of the 128-row tiles
via SBUF->SBUF DMA (engines cannot move data across partitions).

Device layout per core:
  KH  [65,16384] bf16 : rows 0-63 = kh, row 64 = ones
  K2  [128,16384] bf16: rows 0-63 = kl, rows 64-127 = kh (DMA dup)
  QH  [65, 2048] bf16 : rows 0-63 = qh (scale+bias folded), row 64 = -rowmax
  Q2  [128, 2048] bf16: rows 0-63 = qh (DMA dup), rows 64-127 = ql
  V   [128, 128, 65] f32 : 128 token-tiles of [128 tok, 64 v | 1.0]
  pass A: s~[q,kv] chunks (bf16 kh.qh) -> PSUM -> DVE tensor_tensor_reduce
          (pairwise max of 2 chunks/op) -> rowmax -> PE-transpose -> -m
  pass B: sT[kv,q] = mm1+mm2 accumulated in PSUM -> ACT exp -> SBUF (f32r)
  PV:     OT[65,512] += V'.T @ expST  (row 64 = sum = Z)
  final:  PE-transpose OT -> y = O / Z
"""

import sys

sys.path.insert(0, "/opt/trn_rl_repo")

from contextlib import ExitStack

import numpy as np

import concourse.bass as bass
import concourse.mybir as mybir
import concourse.tile as tile
from concourse import bacc
from concourse.bass import ds, ts
from concourse.bass_utils import run_bass_kernel_spmd

N_CORES = 8
N = 16384
D = 64
QR = N // N_CORES          # 2048 q rows per core
N_QTILE = QR // 128        # 16 q tiles per core
N_GROUP = 4                # groups of 4 q-tiles (512 q cols)
TILES_PER_GROUP = N_QTILE // N_GROUP
N_KV_BLK = N // 128        # 128 kv blocks
A_CHUNK = 512              # pass-A kv chunk (1 psum bank)
N_A_CHUNKS = N // A_CHUNK  # 32 chunks per q tile
F32 = mybir.dt.float32
BF16 = mybir.dt.bfloat16
NEG_INF = -3.0e38

_CACHED = {}


def build_kernel():
    nc = bacc.Bacc("TRN2", target_bir_lowering=False, debug=False,
                   num_devices=N_CORES)

    x_d = nc.dram_tensor("x", [N, D], F32, kind="ExternalInput")
    xq_d = nc.dram_tensor("xq", [QR, D], F32, kind="ExternalInput")
    wq_d = nc.dram_tensor("wq", [D + 1, D], F32, kind="ExternalInput")
    wk_d = nc.dram_tensor("wk", [D + 1, D], F32, kind="ExternalInput")
    wv_d = nc.dram_tensor("wv", [D + 1, D], F32, kind="ExternalInput")
    id_d = nc.dram_tensor("ident", [128, 128], F32, kind="ExternalInput")
    y_d = nc.dram_tensor("y", [QR, D], F32, kind="ExternalOutput")

    R32 = mybir.dt.float32r
    MAX = mybir.AluOpType.max

    with tile.TileContext(nc) as tc, ExitStack() as ctx:
        sb = ctx.enter_context(tc.tile_pool(name="sb", bufs=1))
        xtp = ctx.enter_context(tc.tile_pool(name="xtp", bufs=2))
        expp = ctx.enter_context(tc.tile_pool(name="expp", bufs=2))
        smp = ctx.enter_context(tc.tile_pool(name="smp", bufs=4))
        psB_p = ctx.enter_context(tc.tile_pool(name="psB", bufs=1, space="PSUM"))
        psPV_p = ctx.enter_context(tc.tile_pool(name="psPV", bufs=1, space="PSUM"))
        psM_p = ctx.enter_context(tc.tile_pool(name="psM", bufs=1, space="PSUM"))

        # ---- persistent SBUF ----
        KH = sb.tile([D + 1, N], BF16)      # kh | ones row
        K2 = sb.tile([128, N], BF16)        # kl | kh (dup)
        QH = sb.tile([D + 1, QR], BF16)     # qh | -m row
        Q2 = sb.tile([128, QR], BF16)       # qh (dup) | ql
        QL = sb.tile([D, QR], BF16)         # ql staging (partitions 0-63)
        V = sb.tile([128, N_KV_BLK, D + 1], F32)
        OT = sb.tile([D + 1, QR], F32)
        y_sb = sb.tile([128, N_QTILE, D], F32)
        wq = sb.tile([D + 1, D], F32)
        wk = sb.tile([D + 1, D], F32)
        wv = sb.tile([D + 1, D], F32)
        ident = sb.tile([128, 128], F32)
        mms = sb.tile([128, N_QTILE * (N_A_CHUNKS // 2)], F32)  # pair maxes
        scr = sb.tile([128, A_CHUNK], F32)  # ttr elementwise dump (unused)

        def rPV(ap):
            return ap.bitcast(R32)

        nc.gpsimd.dma_start(ident[:], id_d[:])
        nc.gpsimd.dma_start(wq[:], wq_d[:])
        nc.gpsimd.dma_start(wk[:], wk_d[:])
        nc.gpsimd.dma_start(wv[:], wv_d[:])

        nc.vector.memset(V[:, :, D : D + 1], 1.0)

        # ---- PSUM tiles ----
        # Two separate half tiles so the pair-p+2 matmuls don't pick up a
        # false WAR against exp(p) via coarse whole-tile dependency tracking.
        psB0 = psB_p.tile([128, 1024], F32)            # 2 banks
        psB1 = psB_p.tile([128, 1024], F32)            # 2 banks
        psPV = psPV_p.tile([128, 512], F32)            # 1 bank
        # psM pool: tag "psm" (1 bank) + tag "psA" (2 banks) -> 8 total

        def build_xT(xT, dram_ap, ntiles, first_small=False):
            """dram_ap: [ntiles*128, 64] -> xT[0:D, 0:ntiles*128] via PE."""
            splits = list(range(0, ntiles, 16))
            if first_small:
                splits = [0, 4] + [h for h in splits if h >= 16]
            for i, h in enumerate(splits):
                nxt = splits[i + 1] if i + 1 < len(splits) else ntiles
                nh = nxt - h
                xn = xtp.tile([128, 16, D], F32, tag="xn")
                nc.sync.dma_start(
                    xn[:, 0:nh, :],
                    dram_ap[ds(h * 128, nh * 128), :].rearrange(
                        "(j p) d -> p j d", p=128))
                for j4 in range(nh // 4):
                    pm = xpose_psum()
                    for jj in range(4):
                        nc.tensor.transpose(pm[:, ds(jj * 128, 128)],
                                            xn[:, j4 * 4 + jj, :], ident[:])
                    nc.scalar.copy(xT[0:D, ds(h * 128 + j4 * 512, 512)], pm[:])

        # During phase 0 the B-phase and PV psum tiles are idle, so setup
        # borrows them as rings: transpose quads + k/q projections share the
        # four [0:64, 512] psB quarters; V projections rotate psPV's halves.
        # The PE then never stalls on the split/copy reads of the previous
        # setup unit.
        proj_ring = [psB0[0:D, ds(0, 512)], psB0[0:D, ds(512, 512)],
                     psB1[0:D, ds(0, 512)], psB1[0:D, ds(512, 512)]]
        vproj_ring = [psPV[:, ds(0, 256)], psPV[:, ds(256, 256)]]
        ring_i = [0, 0]

        def proj_psum():
            pm = proj_ring[ring_i[0] % 4]
            ring_i[0] += 1
            return pm

        def vproj_psum():
            pm = vproj_ring[ring_i[1] % 2]
            ring_i[1] += 1
            return pm

        xpose_psum = proj_psum

        # ---- setup: xq -> QH/Q2/QL ----
        xqT = xtp.tile([D + 1, QR], F32, tag="xt")
        build_xT(xqT, xq_d[:], N_QTILE, first_small=True)
        nc.gpsimd.memset(xqT[D : D + 1, :], 1.0)
        for j in range(QR // 512):
            pm = proj_psum()
            nc.tensor.matmul(pm[:], wq[:], xqT[:, ts(j, 512)],
                             start=True, stop=True)
            nc.vector.tensor_copy(QH[0:D, ts(j, 512)], pm[:])
            nc.vector.tensor_sub(QL[:, ts(j, 512)], pm[:], QH[0:D, ts(j, 512)])
        nc.gpsimd.dma_start(Q2[0:D, :], QH[0:D, :])
        nc.gpsimd.dma_start(Q2[D : D + D, :], QL[:])

        # ---- setup generator: KH/K2 and V built in 4 chunks of 4096 ----
        def setup_units():
            for cc in range(4):
                xT = xtp.tile([D + 1, 4096], F32, tag="xt")
                build_xT(xT, x_d[ds(cc * 4096, 4096), :], 32)
                nc.gpsimd.memset(xT[D : D + 1, :], 1.0)
                for j in range(8):
                    # k-unit (even positions, so pass-A cov pacing is even)
                    pm = proj_psum()
                    nc.tensor.matmul(pm[:], wk[:], xT[:, ts(j, 512)],
                                     start=True, stop=True)
                    sl = ds(cc * 4096 + j * 512, 512)
                    # kh on ACT, kl on DVE: balances phase-0 engine load
                    nc.scalar.copy(KH[0:D, sl], pm[:])
                    nc.vector.tensor_sub(K2[0:D, sl], pm[:], KH[0:D, sl])
                    yield
                    # v-unit: quad of token-tiles sharing one psum slot
                    pm = vproj_psum()
                    for jj in range(4):
                        nc.tensor.matmul(
                            pm[:, ds(jj * D, D)],
                            xT[:, ts(j * 4 + jj, 128)], wv[:],
                            start=True, stop=True)
                    b0 = cc * 32 + j * 4
                    nc.scalar.copy(rPV(V[:, ds(b0, 4), 0:D]), pm[:])
                    yield
                nc.gpsimd.memset(KH[D : D + 1, ds(cc * 4096, 4096)], 1.0)
                nc.gpsimd.dma_start(K2[D : D + D, ds(cc * 4096, 4096)],
                                  KH[0:D, ds(cc * 4096, 4096)])

        # ---- pass A unit: q-tile t, kv chunk c (bf16 kh.qh, max only) ----
        mfin_pending = []
        a_prev = [None]

        def flush_mfin():
            while mfin_pending:
                t, mt = mfin_pending.pop(0)
                pneg = psM_p.tile([128, A_CHUNK], F32, tag="psA",
                                  bufs=3, name="pneg")[0:1, 0:128]
                nc.tensor.matmul(pneg[:], mt[:], ident[:], start=True, stop=True)
                # QH row 64 <- -m (bf16; per-row constant shift is exact math)
                nc.scalar.mul(QH[D : D + 1, ts(t, 128)], pneg[:], -1.0)

        def emit_A(t, c):
            if c % 8 == 4:
                flush_mfin()
            pa = psM_p.tile([128, A_CHUNK], F32, tag="psA", bufs=3)
            nc.tensor.matmul(pa[:], QH[0:D, ts(t, 128)],
                             KH[0:D, ds(c * A_CHUNK, A_CHUNK)],
                             start=True, stop=True)
            if c % 2 == 0:
                a_prev[0] = pa
            else:
                # one DVE op: elementwise max of both chunks + free-axis max
                nc.vector.tensor_tensor_reduce(
                    out=scr[:], in0=a_prev[0][:], in1=pa[:], scale=1.0,
                    scalar=NEG_INF, op0=MAX, op1=MAX,
                    accum_out=mms[:, t * 16 + c // 2 : t * 16 + c // 2 + 1])
            if c == N_A_CHUNKS - 1:
                mt = smp.tile([128, 1], F32, tag="mt")
                nc.vector.reduce_max(mt[:], mms[:, ts(t, 16)],
                                     axis=mybir.AxisListType.X)
                mfin_pending.append((t, mt))

        # ---- pass B + PV unit: group g, kv block b ----
        pv_pending = []

        def emit_PV_pending(keep=0):
            # PV(pair p) data-depends on exp(p); draining it only two pairs
            # later keeps the in-order PE queue from head-of-line blocking
            # on the ACT exp latency.
            while len(pv_pending) > keep:
                bb, ex_ap = pv_pending.pop(0)
                nc.tensor.matmul(psPV[0 : D + 1, :], rPV(V[:, bb, :]),
                                 rPV(ex_ap),
                                 start=(bb == 0), stop=(bb == N_KV_BLK - 1),
                                 skip_group_check=True)

        def emit_B(g, b):
            half = psB0 if (b // 2) % 2 == 0 else psB1
            off = (b % 2) * 512
            nc.tensor.matmul(half[:, ds(off, 512)], KH[:, ts(b, 128)],
                             QH[:, ds(g * 512, 512)], start=True, stop=False)
            nc.tensor.matmul(half[:, ds(off, 512)], K2[:, ts(b, 128)],
                             Q2[:, ds(g * 512, 512)], start=False, stop=True,
                             skip_group_check=True)
            if b % 2 == 1:
                emit_PV_pending(keep=2)
                ex = expp.tile([128, 1024], F32, tag="ex", bufs=3)
                nc.scalar.activation(rPV(ex[:]), half[:],
                                     mybir.ActivationFunctionType.Exp)
                pv_pending.append((b - 1, ex[:, ds(0, 512)]))
                pv_pending.append((b, ex[:, ds(512, 512)]))

        # ---- main pipeline ----
        # Emission order IS program order for Tile's dependency tracking.
        setup_gen = setup_units()
        setup_done = [0]

        def pace_setup(need):
            while setup_done[0] < need:
                if next(setup_gen, None) is None and setup_done[0] >= 64:
                    break
                setup_done[0] += 1

        final_pending = []

        def emit_final():
            t = final_pending.pop(0)
            pO = psM_p.tile([128, A_CHUNK], F32, tag="psA",
                            bufs=3, name="pO")[:, 0 : D + 1]
            nc.tensor.matmul(pO[:], OT[:, ts(t, 128)],
                             ident[0 : D + 1, 0 : D + 1],
                             start=True, stop=True)
            rz = smp.tile([128, 1], F32, tag="rz")
            nc.vector.reciprocal(rz[:], pO[:, D : D + 1])
            nc.vector.tensor_scalar_mul(y_sb[:, t, :], pO[:, 0:D], rz[:])

        for phase in range(N_GROUP + 1):
            # pair-major A order: both chunks of a ttr pair for tile t, then
            # the next tile -- spreads the DVE max-scan evenly through the
            # phase instead of leaving a scan-bound tail.
            a_units = []
            if phase < N_GROUP:
                for c2 in range(N_A_CHUNKS // 2):
                    for tt in range(TILES_PER_GROUP):
                        t = phase * TILES_PER_GROUP + tt
                        a_units.append((t, 2 * c2, c2 * 4 + tt))
                        a_units.append((t, 2 * c2 + 1, c2 * 4 + tt))
            b_units = []
            if phase > 0:
                b_units = [(phase - 1, b) for b in range(N_KV_BLK)]

            nu = max(len(a_units), len(b_units))
            for u in range(nu):
                if u < len(a_units):
                    t, c, spread = a_units[u]
                    if phase == 0:
                        # setup unit writing KH chunk c is (c//8)*16+2*(c%8);
                        # spread consumes the 64 setup units evenly.
                        cov = (c // 8) * 16 + 2 * (c % 8) + 1
                        pace_setup(max(cov, min(64, spread + 1)))
                    emit_A(t, c)
                if u < len(b_units):
                    emit_B(*b_units[u])
                if final_pending and u >= 8:
                    emit_final()
            flush_mfin()
            if phase == 0:
                pace_setup(64)  # drain remaining V builds
            if phase > 0:
                g = phase - 1
                emit_PV_pending()
                nc.scalar.copy(OT[:, ds(g * 512, 512)], psPV[0 : D + 1, :])
                final_pending.extend(
                    g * TILES_PER_GROUP + tt for tt in range(TILES_PER_GROUP))
                if g > 0:
                    gp = g - 1
                    nc.sync.dma_start(
                        y_d.rearrange("(t p) d -> p t d", p=128)[
                            :, ds(gp * TILES_PER_GROUP, TILES_PER_GROUP), :],
                        y_sb[:, ds(gp * TILES_PER_GROUP, TILES_PER_GROUP), :])

        while final_pending:
            emit_final()
        nc.sync.dma_start(
            y_d.rearrange("(t p) d -> p t d", p=128)[
                :, ds(3 * TILES_PER_GROUP, TILES_PER_GROUP), :],
            y_sb[:, ds(3 * TILES_PER_GROUP, TILES_PER_GROUP), :])

    nc.compile()
    return nc


def _prep_inputs(x, params, Wq, bq, Wk, bk, Wv, bv):
    x = np.ascontiguousarray(x, dtype=np.float32)
    params = np.asarray(params, dtype=np.float32)
    rot = params[:, :D]
    ent = params[:, D : 2 * D]
    scale = np.float32(1.0 / np.sqrt(D))
    wq_eff = (np.asarray(Wq, np.float32) @ rot)
    wk_eff = (np.asarray(Wk, np.float32) @ ent)
    wq = np.vstack([wq_eff.T, np.asarray(bq, np.float32)[None]]) * scale
    wk = np.vstack([wk_eff.T, np.asarray(bk, np.float32)[None]])
    wv = np.vstack([np.asarray(Wv, np.float32).T,
                    np.asarray(bv, np.float32)[None]])
    ident = np.eye(128, dtype=np.float32)
    return x, np.ascontiguousarray(wq), np.ascontiguousarray(wk), \
        np.ascontiguousarray(wv), ident


def kernel(x, params, Wq, bq, Wk, bk, Wv, bv, _trace=False):
    x, wq, wk, wv, ident = _prep_inputs(x, params, Wq, bq, Wk, bk, Wv, bv)
    if "nc" not in _CACHED:
        _CACHED["nc"] = build_kernel()
    nc = _CACHED["nc"]
    in_maps = []
    for c in range(N_CORES):
        in_maps.append({
            "x": x,
            "xq": np.ascontiguousarray(x[c * QR : (c + 1) * QR]),
            "wq": wq, "wk": wk, "wv": wv, "ident": ident,
        })
    res = run_bass_kernel_spmd(nc, in_maps, core_ids=list(range(N_CORES)),
                               trace=_trace)
    out = np.concatenate([res.results[c]["y"] for c in range(N_CORES)], axis=0)
    if _trace:
        _CACHED["last_result"] = res
    global _CACHED_RES
    _CACHED_RES = res
    return out
